# revision 1
# baseline (speedup 1.0000x reference)
"""Attention block (single head) on 8 TRN2 NeuronCores.

Reference (per batch element b):
    Q = x @ Wq; K = x @ Wk; V = x @ Wv          (x: [S, D], W*: [D, D])
    out = softmax(Q @ K^T / sqrt(D)) @ V

Sharding: data-parallel over batch B=8 -> one batch element per core.
No collectives needed; weights are replicated.

All matmul operands are bf16 (PE runs bf16 at 1 row/cycle vs ~2 for
fp32r's HIGH mode on this hw); accumulation stays fp32 in PSUM and the
output is written fp32. Inputs are cast to bf16 on the host so DMA
traffic halves and no on-chip cast pass is needed; x is also
pre-transposed on the host, eliminating all PE transpose work. End-to-end rel err
vs the fp32 reference is ~4e-3 (tolerance 2e-2).

Per-core layout (S=2048, D=512, P=128):
  xt_all [128, 4, 2048]: x^T, DMA'd directly from the host-pre-transposed
      x ([D, S] bf16) — no on-chip transposes at all.
  QT[ei], KT[ei] [128, 2048] = Q^T, K^T  (lhsT=W slice, rhs=xT).
  V_full[si] [128, 2, 258]: V in two 256-halves, a ones column at free
      index 256 of each half (softmax denominator), col 257 zero padding
      (even moving free dim).
  S^T [k, q] chunks = K @ Q^T  (lhsT=KT slice, rhs=QT 512-chunk).
  E^T = exp(S^T / sqrt(D))     (ScalarE, PSUM -> SBUF, bf16 out).
  AV:  psum[q-tile, 258|256] = sum_k E^T-slice @ [V half | 1 | 0]; h=0's
      col 256 is rowsum(E); normalize via DVE reciprocal + tensor_scalar
      mul. h=1 runs bare 256 value columns (no denominator needed twice).
"""

import contextlib

import ml_dtypes
import numpy as np

from concourse import bacc, mybir, tile
from concourse.bass_utils import run_bass_kernel_spmd
from concourse.masks import make_identity

P = 128
S = 2048
D = 512
B = 8
N_CORES = 8
SCALE = float(1.0 / np.sqrt(D))

F32 = mybir.dt.float32
BF16 = mybir.dt.bfloat16

N_ST = S // P    # 16 s-tiles (also k-tiles)
N_DT = D // P    # 4 d-tiles (input dim, also e-tiles)
N_QC = S // 512  # 4 q-chunks of 512


def _emit(nc, tc, x, wq, wk, wv, out):
    ctx = contextlib.ExitStack()
    with ctx:
        wpool = ctx.enter_context(tc.tile_pool(name="wpool", bufs=1))
        persist = ctx.enter_context(tc.tile_pool(name="persist", bufs=1))
        misc = ctx.enter_context(tc.tile_pool(name="misc", bufs=2))
        xtp = ctx.enter_context(tc.tile_pool(name="xt", bufs=1))
        xst = ctx.enter_context(tc.tile_pool(name="xstage", bufs=16))
        etp = ctx.enter_context(tc.tile_pool(name="et", bufs=1))
        ost = ctx.enter_context(tc.tile_pool(name="ostage", bufs=2))
        ps = ctx.enter_context(tc.tile_pool(name="ps", bufs=1, space="PSUM"))

        x_engines = (nc.sync, nc.scalar)
        ones2 = misc.tile([P, 2, 2], BF16, tag="ones2")
        nc.vector.memset(ones2[:, :, :], 0.0)
        nc.vector.memset(ones2[:, :, 0:1], 1.0)

        # x arrives from the host already transposed ([D, S] bf16), so
        # x^T slices DMA straight into xt_all: no PE transposes, no
        # identity, no PSUM transpose traffic, no DVE evacuations.
        # Slices interleave with the weight slices across the two
        # hardware DGE queues in consumption order (chunk 0 + wq first).
        def stage_w(wname):
            return wpool.tile([P, N_DT, D], BF16, tag=wname, name=wname)

        def stage_w_half(wt, w_dram, h, eng):
            # half a weight (256 e-cols) per DMA: two completion slots
            # per weight instead of four
            es = slice(h * 256, (h + 1) * 256)
            eng.dma_start(
                wt[:, :, es],
                w_dram.rearrange("(a p) e -> p a e", p=P)[:, :, es],
            )

        wq_t = stage_w("wq")
        wk_t = stage_w("wk")
        wv_t = stage_w("wv")
        xt_all = xtp.tile([P, N_DT, S], BF16, tag="xt_all")

        def stage_xt_chunk(c, eng):
            # one whole 512-col x^T chunk (all 4 d-slices) per DMA: the
            # first projection then waits on THREE sync completions
            # (xt-c0, wq halves) instead of five
            cs = slice(c * 512, (c + 1) * 512)
            eng.dma_start(
                xt_all[:, :, cs],
                x[:, cs].rearrange("(a p) s -> p a s", p=P),
            )

        # wqA rides GpSimd ahead of wv (which has ~5us of slack): it
        # lands ~11-14us there, vs ~15us stuck behind xt-c0's long
        # 512KB slot on Sync — the first projection's binding wait
        stage_xt_chunk(0, nc.sync)
        stage_w_half(wq_t, wq, 0, nc.gpsimd)
        stage_w_half(wk_t, wk, 0, nc.scalar)
        stage_w_half(wq_t, wq, 1, nc.sync)
        stage_w_half(wk_t, wk, 1, nc.scalar)
        stage_xt_chunk(1, nc.scalar)
        nc.gpsimd.dma_start(
            wv_t[:, :, :], wv.rearrange("(a p) e -> p a e", p=P)
        )
        stage_xt_chunk(2, nc.sync)
        stage_xt_chunk(3, nc.scalar)
        wq_sb = [wq_t[:, di, :] for di in range(N_DT)]
        wk_sb = [wk_t[:, di, :] for di in range(N_DT)]
        wv_sb = [wv_t[:, di, :] for di in range(N_DT)]

        qt_sb = [persist.tile([P, S], BF16, tag=f"qt{ei}", name=f"qt{ei}") for ei in range(N_DT)]
        kt_sb = [persist.tile([P, S], BF16, tag=f"kt{ei}", name=f"kt{ei}") for ei in range(N_DT)]
        v_sb = [persist.tile([P, 2, 258], BF16, tag=f"v{si}", name=f"v{si}") for si in range(N_ST)]

        for si in range(N_ST):
            nc.vector.tensor_copy(v_sb[si][:, :, 256:258], ones2[:, :, :])

        # ---------- phase 1: project Q^T, K^T, V from pre-transposed x ----------
        for sc in range(N_QC):
            cs = slice(sc * 512, (sc + 1) * 512)

            def emit_q(ei):
                es = slice(ei * P, (ei + 1) * P)
                pq = ps.tile([P, 512], F32, tag="mm512", bufs=5, name=f"pq{sc}_{ei}")
                for di in range(N_DT):
                    nc.tensor.matmul(
                        pq[:, :], wq_sb[di][:, es], xt_all[:, di, cs],
                        start=(di == 0), stop=(di == N_DT - 1),
                    )
                nc.scalar.copy(qt_sb[ei][:, cs], pq[:, :])

            def emit_k(ei):
                es = slice(ei * P, (ei + 1) * P)
                pk = ps.tile([P, 512], F32, tag="mm512", bufs=5, name=f"pk{sc}_{ei}")
                for di in range(N_DT):
                    nc.tensor.matmul(
                        pk[:, :], wk_sb[di][:, es], xt_all[:, di, cs],
                        start=(di == 0), stop=(di == N_DT - 1),
                    )
                nc.vector.tensor_copy(kt_sb[ei][:, cs], pk[:, :])

            if sc == 0:
                # Q chunks first: pushes the first wk deadline ~2.6us out,
                # so a slow first completion on the Scalar DGE queue
                # (which carries the wk slices) can't stall the PE
                for ei in range(N_DT):
                    emit_q(ei)
                for ei in range(N_DT):
                    emit_k(ei)
            else:
                for ei in range(N_DT):
                    emit_q(ei)
                    emit_k(ei)

            for si in range(sc * 4, sc * 4 + 4):
                ss = slice(si * P, (si + 1) * P)
                pv = ps.tile([P, D], F32, tag="mm512", bufs=5, name=f"pv{si}")
                for di in range(N_DT):
                    nc.tensor.matmul(
                        pv[:, :], xt_all[:, di, ss], wv_sb[di][:, :],
                        start=(di == 0), stop=(di == N_DT - 1),
                    )
                nc.scalar.copy(
                    v_sb[si][:, :, 0:256],
                    pv[:, :].rearrange("p (a b) -> p a b", a=2),
                )

        # ---------- phase 2: scores, softmax, AV ----------
        # et tiles are double-buffered by q-chunk and the emission order
        # is scores(0), scores(1), AV(0), scores(2), AV(1), ... so
        # ScalarE computes the next chunk's exps while the PE runs the
        # previous chunk's AV chains — the PE never waits on the exp
        # pipeline at a chunk boundary.
        def emit_scores(qc):
            qs_all = slice(qc * 512, (qc + 1) * 512)
            et_sb = []
            for ki in range(N_ST):
                ks = slice(ki * P, (ki + 1) * P)
                pst = ps.tile([P, 512], F32, tag="mm512", bufs=5, name=f"pst{qc}_{ki}")
                for ei in range(N_DT):
                    nc.tensor.matmul(
                        pst[:, :], kt_sb[ei][:, ks], qt_sb[ei][:, qs_all],
                        start=(ei == 0), stop=(ei == N_DT - 1),
                    )
                et = etp.tile(
                    [P, 512], BF16, tag=f"et{qc % 2}_{ki}", name=f"et{qc}_{ki}"
                )
                nc.scalar.activation(
                    et[:, :], pst[:, :],
                    mybir.ActivationFunctionType.Exp, scale=SCALE,
                )
                et_sb.append(et)
            return et_sb

        def emit_av(qc, et_sb):
            for qs in range(4):
                qi = qc * 4 + qs
                o_tile = ost.tile([P, D], F32, tag="o", name=f"o{qi}")
                r_sb = misc.tile([P, 1], F32, tag="r", name=f"r{qi}")
                for h in range(2):
                    # only h=0 carries the denominator ones-column; h=1
                    # runs the bare 256 value columns
                    w_av = 258 if h == 0 else 256
                    pav = ps.tile([P, w_av], F32, tag="tpav", bufs=3, name=f"pav{qi}_{h}")
                    for ki in range(N_ST):
                        nc.tensor.matmul(
                            pav[:, :],
                            et_sb[ki][:, qs * P:(qs + 1) * P],
                            v_sb[ki][:, h, 0:w_av],
                            start=(ki == 0), stop=(ki == N_ST - 1),
                        )
                    if h == 0:
                        nc.vector.reciprocal(r_sb[:, :], pav[:, 256:257])
                    nc.vector.tensor_scalar_mul(
                        o_tile[:, h * 256:(h + 1) * 256],
                        pav[:, 0:256],
                        r_sb[:, :],
                    )
                    x_engines[h].dma_start(
                        out[qi * P:(qi + 1) * P, h * 256:(h + 1) * 256],
                        o_tile[:, h * 256:(h + 1) * 256],
                    )

        prev = emit_scores(0)
        for qc in range(1, N_QC):
            cur = emit_scores(qc)
            emit_av(qc - 1, prev)
            prev = cur
        emit_av(N_QC - 1, prev)


_CACHED_NC = None


def _build():
    global _CACHED_NC
    if _CACHED_NC is not None:
        return _CACHED_NC
    nc = bacc.Bacc(
        "TRN2", target_bir_lowering=False, debug=False, num_devices=N_CORES
    )
    x = nc.declare_dram_parameter("x", [D, S], BF16, isOutput=False)
    wq = nc.declare_dram_parameter("wq", [D, D], BF16, isOutput=False)
    wk = nc.declare_dram_parameter("wk", [D, D], BF16, isOutput=False)
    wv = nc.declare_dram_parameter("wv", [D, D], BF16, isOutput=False)
    out = nc.declare_dram_parameter("out", [S, D], F32, isOutput=True)
    with tile.TileContext(nc) as tc:
        _emit(nc, tc, x.ap(), wq.ap(), wk.ap(), wv.ap(), out.ap())
    nc.compile()
    _CACHED_NC = nc
    return nc


def _in_maps(x, Wq, Wk, Wv):
    bf = ml_dtypes.bfloat16
    x = np.ascontiguousarray(np.asarray(x).transpose(0, 2, 1)).astype(bf)
    Wq = np.ascontiguousarray(np.asarray(Wq)).astype(bf)
    Wk = np.ascontiguousarray(np.asarray(Wk)).astype(bf)
    Wv = np.ascontiguousarray(np.asarray(Wv)).astype(bf)
    return [
        {"x": x[b], "wq": Wq, "wk": Wk, "wv": Wv} for b in range(B)
    ]


def kernel(x, Wq, Wk, Wv, **_ignored):
    nc = _build()
    in_maps = _in_maps(x, Wq, Wk, Wv)
    res = run_bass_kernel_spmd(
        nc, in_maps, core_ids=list(range(N_CORES)), trace=False
    )
    return np.stack([res.results[b]["out"] for b in range(B)], axis=0)



# revision 4
# speedup vs baseline: 1.0723x; 1.0723x over previous
"""Attention block (single head) on 8 TRN2 NeuronCores.

Reference (per batch element b):
    Q = x @ Wq; K = x @ Wk; V = x @ Wv          (x: [S, D], W*: [D, D])
    out = softmax(Q @ K^T / sqrt(D)) @ V

Sharding: data-parallel over batch B=8 -> one batch element per core.
No collectives needed; weights are replicated.

v2 algebraic restructure: scores = Q K^T = x (Wq Wk^T) x^T. The host
computes M = Wq @ Wk^T in fp32 (tiny, 512^3) and ships M instead of
Wq/Wk. On-core this kills the whole K projection (1/3 of phase-1 PE
time): only G^T = M^T x^T and V = x Wv are projected, and the scores
matmul contracts G against x^T slices already resident in SBUF.

All matmul operands are bf16 (PE runs bf16 at 1 row/cycle); accumulation
stays fp32 in PSUM and the output is written fp32. Inputs are cast to
bf16 on the host so DMA traffic halves; x is also pre-transposed on the
host, eliminating all PE transpose work. End-to-end rel err vs the fp32
reference is ~3.8e-3 (tolerance 2e-2).

Per-core layout (S=2048, D=512, P=128):
  xt_all [128, 4, 2048]: x^T, DMA'd directly from the host-pre-transposed
      x ([D, S] bf16) — no on-chip transposes at all. Reused as the
      scores lhsT (contraction runs over the d' features of M).
  GT[di] [128, 2048] = G^T  (lhsT=M slice, rhs=xT).
  V_full[si] [128, 2, 258]: V in two 256-halves, a ones column at free
      index 256 of each half (softmax denominator), col 257 zero padding
      (even moving free dim).
  S^T [k, q] chunks = x @ G^T  (lhsT=xT k-slice, rhs=GT 512-chunk).
  E^T = exp(S^T / sqrt(D))     (ScalarE, PSUM -> SBUF, bf16 out).
  AV:  psum[q-tile, 258|256] = sum_k E^T-slice @ [V half | 1 | 0]; h=0's
      col 256 is rowsum(E); normalize via DVE reciprocal + tensor_scalar
      mul. h=1 runs bare 256 value columns (no denominator needed twice).

HAM warmup: the PE clock-gate starts at 1.2 GHz and only reaches 2.4 GHz
after ~3.4us of sustained matmul activity. The first real matmul can't
start until ~11us (input DMA), so ~36 dummy N=128 matmuls on a memset
tile run during the DMA wait to flip the clock gate early.
"""

import contextlib

import ml_dtypes
import numpy as np

from concourse import bacc, mybir, tile
from concourse.bass_utils import run_bass_kernel_spmd

P = 128
S = 2048
D = 512
B = 8
N_CORES = 8
SCALE = float(1.0 / np.sqrt(D))

F32 = mybir.dt.float32
BF16 = mybir.dt.bfloat16

N_ST = S // P    # 16 s-tiles (also k-tiles)
N_DT = D // P    # 4 d-tiles (input dim, also d'-tiles)
N_QC = S // 512  # 4 q-chunks of 512

WARMUP_MM = 36   # dummy N=128 matmuls to flip the HAM clock gate early


def _emit(nc, tc, x, wm, wv, out):
    ctx = contextlib.ExitStack()
    with ctx:
        wpool = ctx.enter_context(tc.tile_pool(name="wpool", bufs=1))
        persist = ctx.enter_context(tc.tile_pool(name="persist", bufs=1))
        misc = ctx.enter_context(tc.tile_pool(name="misc", bufs=2))
        xtp = ctx.enter_context(tc.tile_pool(name="xt", bufs=1))
        etp = ctx.enter_context(tc.tile_pool(name="et", bufs=1))
        ost = ctx.enter_context(tc.tile_pool(name="ostage", bufs=2))
        ps = ctx.enter_context(tc.tile_pool(name="ps", bufs=1, space="PSUM"))

        x_engines = (nc.sync, nc.scalar)
        ones2 = misc.tile([P, 2, 2], BF16, tag="ones2")
        nc.vector.memset(ones2[:, :, :], 0.0)
        nc.vector.memset(ones2[:, :, 0:1], 1.0)

        # PE warmup: dummy matmuls on a memset tile while input DMA is in
        # flight. They rotate through the same psum buffers as the real
        # matmuls (no extra PSUM bank) and finish before the first real
        # matmul's inputs land.
        warm_sb = misc.tile([P, P], BF16, tag="warm")
        nc.vector.memset(warm_sb[:, :], 0.0)
        for i in range(WARMUP_MM):
            pw = ps.tile([P, 512], F32, tag="mm512", bufs=5, name=f"warm{i}")
            nc.tensor.matmul(pw[:, 0:P], warm_sb[:, :], warm_sb[:, :],
                             start=True, stop=True)

        def stage_w(wname):
            return wpool.tile([P, N_DT, D], BF16, tag=wname, name=wname)

        def stage_w_half(wt, w_dram, h, eng):
            # half a weight (256 e-cols) per DMA: two completion slots
            # per weight instead of four
            es = slice(h * 256, (h + 1) * 256)
            eng.dma_start(
                wt[:, :, es],
                w_dram.rearrange("(a p) e -> p a e", p=P)[:, :, es],
            )

        wm_t = stage_w("wm")
        wv_t = stage_w("wv")
        xt_all = xtp.tile([P, N_DT, S], BF16, tag="xt_all")

        def stage_xt_chunk(c, eng):
            cs = slice(c * 512, (c + 1) * 512)
            eng.dma_start(
                xt_all[:, :, cs],
                x[:, cs].rearrange("(a p) s -> p a s", p=P),
            )

        # First G-projection group (ei=0,1) needs xt-c0 + wm half A; ei=2,3
        # additionally need half B. Spread the three prerequisites over
        # three different engine DGE queues so none serializes behind
        # another.
        stage_xt_chunk(0, nc.sync)
        stage_w_half(wm_t, wm, 0, nc.gpsimd)
        stage_w_half(wm_t, wm, 1, nc.scalar)
        stage_xt_chunk(1, nc.scalar)
        nc.gpsimd.dma_start(
            wv_t[:, :, :], wv.rearrange("(a p) e -> p a e", p=P)
        )
        stage_xt_chunk(2, nc.sync)
        stage_xt_chunk(3, nc.scalar)
        wm_sb = [wm_t[:, di, :] for di in range(N_DT)]
        wv_sb = [wv_t[:, di, :] for di in range(N_DT)]

        gt_sb = [persist.tile([P, S], BF16, tag=f"gt{di}", name=f"gt{di}") for di in range(N_DT)]
        v_sb = [persist.tile([P, 2, 258], BF16, tag=f"v{si}", name=f"v{si}") for si in range(N_ST)]

        for si in range(N_ST):
            nc.vector.tensor_copy(v_sb[si][:, :, 256:258], ones2[:, :, :])

        # ---------- phase 1: project G^T and V from pre-transposed x ----------
        for sc in range(N_QC):
            cs = slice(sc * 512, (sc + 1) * 512)

            for ei in range(N_DT):
                es = slice(ei * P, (ei + 1) * P)
                pg = ps.tile([P, 512], F32, tag="mm512", bufs=5, name=f"pg{sc}_{ei}")
                for di in range(N_DT):
                    nc.tensor.matmul(
                        pg[:, :], wm_sb[di][:, es], xt_all[:, di, cs],
                        start=(di == 0), stop=(di == N_DT - 1),
                    )
                nc.scalar.copy(gt_sb[ei][:, cs], pg[:, :])

            for si in range(sc * 4, sc * 4 + 4):
                ss = slice(si * P, (si + 1) * P)
                pv = ps.tile([P, D], F32, tag="mm512", bufs=5, name=f"pv{si}")
                for di in range(N_DT):
                    nc.tensor.matmul(
                        pv[:, :], xt_all[:, di, ss], wv_sb[di][:, :],
                        start=(di == 0), stop=(di == N_DT - 1),
                    )
                nc.scalar.copy(
                    v_sb[si][:, :, 0:256],
                    pv[:, :].rearrange("p (a b) -> p a b", a=2),
                )

        # ---------- phase 2: scores, softmax, AV ----------
        # et tiles are double-buffered by q-chunk and the emission order
        # is scores(0), scores(1), AV(0), scores(2), AV(1), ... so
        # ScalarE computes the next chunk's exps while the PE runs the
        # previous chunk's AV chains — the PE never waits on the exp
        # pipeline at a chunk boundary.
        def emit_scores(qc):
            qs_all = slice(qc * 512, (qc + 1) * 512)
            et_sb = []
            for ki in range(N_ST):
                ks = slice(ki * P, (ki + 1) * P)
                pst = ps.tile([P, 512], F32, tag="mm512", bufs=5, name=f"pst{qc}_{ki}")
                for di in range(N_DT):
                    nc.tensor.matmul(
                        pst[:, :], xt_all[:, di, ks], gt_sb[di][:, qs_all],
                        start=(di == 0), stop=(di == N_DT - 1),
                    )
                et = etp.tile(
                    [P, 512], BF16, tag=f"et{qc % 2}_{ki}", name=f"et{qc}_{ki}"
                )
                nc.scalar.activation(
                    et[:, :], pst[:, :],
                    mybir.ActivationFunctionType.Exp, scale=SCALE,
                )
                et_sb.append(et)
            return et_sb

        def emit_av(qc, et_sb):
            for qs in range(4):
                qi = qc * 4 + qs
                o_tile = ost.tile([P, D], F32, tag="o", name=f"o{qi}")
                r_sb = misc.tile([P, 1], F32, tag="r", name=f"r{qi}")
                for h in range(2):
                    # only h=0 carries the denominator ones-column; h=1
                    # runs the bare 256 value columns
                    w_av = 258 if h == 0 else 256
                    pav = ps.tile([P, w_av], F32, tag="tpav", bufs=3, name=f"pav{qi}_{h}")
                    for ki in range(N_ST):
                        nc.tensor.matmul(
                            pav[:, :],
                            et_sb[ki][:, qs * P:(qs + 1) * P],
                            v_sb[ki][:, h, 0:w_av],
                            start=(ki == 0), stop=(ki == N_ST - 1),
                        )
                    if h == 0:
                        nc.vector.reciprocal(r_sb[:, :], pav[:, 256:257])
                    nc.vector.tensor_scalar_mul(
                        o_tile[:, h * 256:(h + 1) * 256],
                        pav[:, 0:256],
                        r_sb[:, :],
                    )
                    x_engines[h].dma_start(
                        out[qi * P:(qi + 1) * P, h * 256:(h + 1) * 256],
                        o_tile[:, h * 256:(h + 1) * 256],
                    )

        prev = emit_scores(0)
        for qc in range(1, N_QC):
            cur = emit_scores(qc)
            emit_av(qc - 1, prev)
            prev = cur
        emit_av(N_QC - 1, prev)


_CACHED_NC = None


def _build():
    global _CACHED_NC
    if _CACHED_NC is not None:
        return _CACHED_NC
    nc = bacc.Bacc(
        "TRN2", target_bir_lowering=False, debug=False, num_devices=N_CORES
    )
    x = nc.declare_dram_parameter("x", [D, S], BF16, isOutput=False)
    wm = nc.declare_dram_parameter("wm", [D, D], BF16, isOutput=False)
    wv = nc.declare_dram_parameter("wv", [D, D], BF16, isOutput=False)
    out = nc.declare_dram_parameter("out", [S, D], F32, isOutput=True)
    with tile.TileContext(nc) as tc:
        _emit(nc, tc, x.ap(), wm.ap(), wv.ap(), out.ap())
    nc.compile()
    _CACHED_NC = nc
    return nc


def _in_maps(x, Wq, Wk, Wv):
    bf = ml_dtypes.bfloat16
    x = np.ascontiguousarray(np.asarray(x).transpose(0, 2, 1)).astype(bf)
    M = (np.asarray(Wq, dtype=np.float32) @ np.asarray(Wk, dtype=np.float32).T).astype(bf)
    Wv = np.ascontiguousarray(np.asarray(Wv)).astype(bf)
    return [
        {"x": x[b], "wm": M, "wv": Wv} for b in range(B)
    ]


def kernel(x, Wq, Wk, Wv, **_ignored):
    nc = _build()
    in_maps = _in_maps(x, Wq, Wk, Wv)
    res = run_bass_kernel_spmd(
        nc, in_maps, core_ids=list(range(N_CORES)), trace=False
    )
    return np.stack([res.results[b]["out"] for b in range(B)], axis=0)


# revision 5
# speedup vs baseline: 1.0821x; 1.0091x over previous
"""Attention block (single head) on 8 TRN2 NeuronCores.

Reference (per batch element b):
    Q = x @ Wq; K = x @ Wk; V = x @ Wv          (x: [S, D], W*: [D, D])
    out = softmax(Q @ K^T / sqrt(D)) @ V

Sharding: data-parallel over batch B=8 -> one batch element per core.
No collectives needed; weights are replicated.

v2 algebraic restructure: scores = Q K^T = x (Wq Wk^T) x^T. The host
computes M = Wq @ Wk^T in fp32 (tiny, 512^3) and ships M instead of
Wq/Wk. On-core this kills the whole K projection (1/3 of phase-1 PE
time): only G^T = M^T x^T and V = x Wv are projected, and the scores
matmul contracts G against x^T slices already resident in SBUF.

v3 DMA restructure: HBM DMA here is descriptor-bound (~215ns per
descriptor; a descriptor covers one contiguous DRAM line). The host
pre-arranges every input so each SBUF partition row is one contiguous
4KB DRAM line: x ships as [chunk, 128, 4*512] (chunk-major), weights as
[128, 4*512]. Output merges the two 256-col halves into one DMA per
q-tile (2KB lines) issued on the otherwise-idle GpSimd (SWDGE) ring.

All matmul operands are bf16; accumulation stays fp32 in PSUM and the
output is written fp32. End-to-end rel err vs the fp32 reference is
~3.8e-3 (tolerance 2e-2).

Per-core layout (S=2048, D=512, P=128):
  xt_all [128, 4, 4, 512]: x^T as [p, chunk, dtile, s'], DMA'd directly
      from the host-pre-transposed x. Reused as the scores lhsT
      (contraction runs over the d' features of M).
  GT[di] [128, 2048] = G^T  (lhsT=M slice, rhs=xT).
  V_full[si] [128, 2, 258]: V in two 256-halves, a ones column at free
      index 256 of each half (softmax denominator), col 257 zero padding.
  S^T [k, q] chunks = x @ G^T  (lhsT=xT k-slice, rhs=GT 512-chunk).
  E^T = exp(S^T / sqrt(D))     (ScalarE, PSUM -> SBUF, bf16 out).
  AV:  psum[q-tile, 258|256] = sum_k E^T-slice @ [V half | 1 | 0]; h=0's
      col 256 is rowsum(E); normalize via DVE reciprocal + tensor_scalar
      mul into a [128, 512] staging tile, one DMA per q-tile.

HAM warmup: the PE clock-gate starts at 1.2 GHz and only reaches 2.4 GHz
after ~3.4us of sustained matmul activity. Dummy N=128 matmuls on a
memset tile run during the input DMA wait to flip the clock gate early.
"""

import contextlib

import ml_dtypes
import numpy as np

from concourse import bacc, mybir, tile
from concourse.bass_utils import run_bass_kernel_spmd

P = 128
S = 2048
D = 512
B = 8
N_CORES = 8
SCALE = float(1.0 / np.sqrt(D))

F32 = mybir.dt.float32
BF16 = mybir.dt.bfloat16

N_ST = S // P    # 16 s-tiles (also k-tiles)
N_DT = D // P    # 4 d-tiles (input dim, also d'-tiles)
N_QC = S // 512  # 4 q-chunks of 512

WARMUP_MM = 20   # dummy N=128 matmuls to flip the HAM clock gate early


def _xt(xt_all, di, s0, s1):
    """Slice x^T [128, d-block di, global s range) out of the chunked layout."""
    c0, o0 = divmod(s0, 512)
    c1, o1 = divmod(s1 - 1, 512)
    assert c0 == c1, (s0, s1)
    return xt_all[:, c0, di, o0:o1 + 1]


def _emit(nc, tc, x, wm, wv, out):
    ctx = contextlib.ExitStack()
    with ctx:
        wpool = ctx.enter_context(tc.tile_pool(name="wpool", bufs=1))
        persist = ctx.enter_context(tc.tile_pool(name="persist", bufs=1))
        misc = ctx.enter_context(tc.tile_pool(name="misc", bufs=2))
        xtp = ctx.enter_context(tc.tile_pool(name="xt", bufs=1))
        etp = ctx.enter_context(tc.tile_pool(name="et", bufs=1))
        ost = ctx.enter_context(tc.tile_pool(name="ostage", bufs=2))
        ps = ctx.enter_context(tc.tile_pool(name="ps", bufs=1, space="PSUM"))

        ones2 = misc.tile([P, 2, 2], BF16, tag="ones2")
        nc.vector.memset(ones2[:, :, :], 0.0)
        nc.vector.memset(ones2[:, :, 0:1], 1.0)

        # PE warmup: dummy matmuls on a memset tile while input DMA is in
        # flight; they rotate through the real psum buffers.
        warm_sb = misc.tile([P, P], BF16, tag="warm")
        nc.vector.memset(warm_sb[:, :], 0.0)
        for i in range(WARMUP_MM):
            pw = ps.tile([P, 512], F32, tag="mm512", bufs=5, name=f"warm{i}")
            nc.tensor.matmul(pw[:, 0:P], warm_sb[:, :], warm_sb[:, :],
                             start=True, stop=True)

        # Inputs arrive in host-prearranged layouts: every partition row is
        # one contiguous 4KB DRAM line (descriptor-efficient).
        wm_t = wpool.tile([P, N_DT, D], BF16, tag="wm", name="wm")
        wv_t = wpool.tile([P, N_DT, D], BF16, tag="wv", name="wv")
        xt_all = xtp.tile([P, N_QC, N_DT, 512], BF16, tag="xt_all")

        # First G-projection group needs xt-c0 (sync ring) + wm (scalar
        # ring); wv (gpsimd ring) is needed one group later. The three
        # rings drain in parallel.
        nc.sync.dma_start(
            xt_all[:, 0, :, :], x[0].rearrange("p (a s) -> p a s", a=N_DT)
        )
        nc.scalar.dma_start(
            wm_t[:, :, :], wm.rearrange("p (a e) -> p a e", a=N_DT)
        )
        nc.gpsimd.dma_start(
            wv_t[:, :, :], wv.rearrange("p (a e) -> p a e", a=N_DT)
        )
        nc.sync.dma_start(
            xt_all[:, 1, :, :], x[1].rearrange("p (a s) -> p a s", a=N_DT)
        )
        nc.scalar.dma_start(
            xt_all[:, 2, :, :], x[2].rearrange("p (a s) -> p a s", a=N_DT)
        )
        nc.sync.dma_start(
            xt_all[:, 3, :, :], x[3].rearrange("p (a s) -> p a s", a=N_DT)
        )
        wm_sb = [wm_t[:, di, :] for di in range(N_DT)]
        wv_sb = [wv_t[:, di, :] for di in range(N_DT)]

        gt_sb = [persist.tile([P, S], BF16, tag=f"gt{di}", name=f"gt{di}") for di in range(N_DT)]
        v_sb = [persist.tile([P, 2, 258], BF16, tag=f"v{si}", name=f"v{si}") for si in range(N_ST)]

        for si in range(N_ST):
            nc.vector.tensor_copy(v_sb[si][:, :, 256:258], ones2[:, :, :])

        # ---------- phase 1: project G^T and V from pre-transposed x ----------
        for sc in range(N_QC):
            cs = slice(sc * 512, (sc + 1) * 512)

            for ei in range(N_DT):
                es = slice(ei * P, (ei + 1) * P)
                pg = ps.tile([P, 512], F32, tag="mm512", bufs=5, name=f"pg{sc}_{ei}")
                for di in range(N_DT):
                    nc.tensor.matmul(
                        pg[:, :], wm_sb[di][:, es], xt_all[:, sc, di, :],
                        start=(di == 0), stop=(di == N_DT - 1),
                    )
                nc.scalar.copy(gt_sb[ei][:, cs], pg[:, :])

            for si in range(sc * 4, sc * 4 + 4):
                pv = ps.tile([P, D], F32, tag="mm512", bufs=5, name=f"pv{si}")
                for di in range(N_DT):
                    nc.tensor.matmul(
                        pv[:, :], _xt(xt_all, di, si * P, (si + 1) * P), wv_sb[di][:, :],
                        start=(di == 0), stop=(di == N_DT - 1),
                    )
                nc.scalar.copy(
                    v_sb[si][:, :, 0:256],
                    pv[:, :].rearrange("p (a b) -> p a b", a=2),
                )

        # ---------- phase 2: scores, softmax, AV ----------
        # et tiles are double-buffered by q-chunk and the emission order
        # is scores(0), scores(1), AV(0), scores(2), AV(1), ... so
        # ScalarE computes the next chunk's exps while the PE runs the
        # previous chunk's AV chains.
        def emit_scores(qc):
            qs_all = slice(qc * 512, (qc + 1) * 512)
            et_sb = []
            for ki in range(N_ST):
                pst = ps.tile([P, 512], F32, tag="mm512", bufs=5, name=f"pst{qc}_{ki}")
                for di in range(N_DT):
                    nc.tensor.matmul(
                        pst[:, :], _xt(xt_all, di, ki * P, (ki + 1) * P), gt_sb[di][:, qs_all],
                        start=(di == 0), stop=(di == N_DT - 1),
                    )
                et = etp.tile(
                    [P, 512], BF16, tag=f"et{qc % 2}_{ki}", name=f"et{qc}_{ki}"
                )
                nc.scalar.activation(
                    et[:, :], pst[:, :],
                    mybir.ActivationFunctionType.Exp, scale=SCALE,
                )
                et_sb.append(et)
            return et_sb

        def emit_av(qc, et_sb):
            for qs in range(4):
                qi = qc * 4 + qs
                o_tile = ost.tile([P, D], F32, tag="o", name=f"o{qi}")
                r_sb = misc.tile([P, 1], F32, tag="r", name=f"r{qi}")
                for h in range(2):
                    # only h=0 carries the denominator ones-column; h=1
                    # runs the bare 256 value columns
                    w_av = 258 if h == 0 else 256
                    pav = ps.tile([P, w_av], F32, tag="tpav", bufs=3, name=f"pav{qi}_{h}")
                    for ki in range(N_ST):
                        nc.tensor.matmul(
                            pav[:, :],
                            et_sb[ki][:, qs * P:(qs + 1) * P],
                            v_sb[ki][:, h, 0:w_av],
                            start=(ki == 0), stop=(ki == N_ST - 1),
                        )
                    if h == 0:
                        nc.vector.reciprocal(r_sb[:, :], pav[:, 256:257])
                    nc.vector.tensor_scalar_mul(
                        o_tile[:, h * 256:(h + 1) * 256],
                        pav[:, 0:256],
                        r_sb[:, :],
                    )
                # one DMA per q-tile (2KB contiguous DRAM lines) on the
                # otherwise-idle SWDGE ring
                nc.gpsimd.dma_start(
                    out[qi * P:(qi + 1) * P, :], o_tile[:, :]
                )

        prev = emit_scores(0)
        for qc in range(1, N_QC):
            cur = emit_scores(qc)
            emit_av(qc - 1, prev)
            prev = cur
        emit_av(N_QC - 1, prev)


_CACHED_NC = None


def _build():
    global _CACHED_NC
    if _CACHED_NC is not None:
        return _CACHED_NC
    nc = bacc.Bacc(
        "TRN2", target_bir_lowering=False, debug=False, num_devices=N_CORES
    )
    x = nc.declare_dram_parameter("x", [N_QC, P, N_DT * 512], BF16, isOutput=False)
    wm = nc.declare_dram_parameter("wm", [P, N_DT * D], BF16, isOutput=False)
    wv = nc.declare_dram_parameter("wv", [P, N_DT * D], BF16, isOutput=False)
    out = nc.declare_dram_parameter("out", [S, D], F32, isOutput=True)
    with tile.TileContext(nc) as tc:
        _emit(nc, tc, x.ap(), wm.ap(), wv.ap(), out.ap())
    nc.compile()
    _CACHED_NC = nc
    return nc


def _host_w(w):
    # [d, e] with d = a*128 + p  ->  [p, a*e] (each partition row 4KB)
    return np.ascontiguousarray(
        np.asarray(w, dtype=np.float32).reshape(N_DT, P, D).transpose(1, 0, 2)
        .reshape(P, N_DT * D)
    ).astype(ml_dtypes.bfloat16)


def _in_maps(x, Wq, Wk, Wv):
    bf = ml_dtypes.bfloat16
    # x [B, S, D] -> x^T [D, S] -> [c, p, a*s'] chunk-major, 4KB lines
    xt = np.asarray(x, dtype=np.float32).transpose(0, 2, 1)  # [B, D, S]
    xh = np.ascontiguousarray(
        xt.reshape(B, N_DT, P, N_QC, 512).transpose(0, 3, 2, 1, 4)
        .reshape(B, N_QC, P, N_DT * 512)
    ).astype(bf)
    M = np.asarray(Wq, dtype=np.float32) @ np.asarray(Wk, dtype=np.float32).T
    wm = _host_w(M)
    wv = _host_w(Wv)
    return [
        {"x": xh[b], "wm": wm, "wv": wv} for b in range(B)
    ]


def kernel(x, Wq, Wk, Wv, **_ignored):
    nc = _build()
    in_maps = _in_maps(x, Wq, Wk, Wv)
    res = run_bass_kernel_spmd(
        nc, in_maps, core_ids=list(range(N_CORES)), trace=False
    )
    return np.stack([res.results[b]["out"] for b in range(B)], axis=0)
